# revision 10
# baseline (speedup 1.0000x reference)
"""Distance-loss kernel for Trainium2 (8 NeuronCores, SPMD data-parallel).

loss = sum_{b,c,h} || output[b,c,h,:] - target[b,c,h,:] + eps ||_2

Strategy: flatten both (16,8,512,512) f32 inputs to rows of W=512
(B*C*H = 65536 rows), shard rows contiguously across 8 cores (8192
rows/core).  Each core streams its 2 x 16 MiB in [128, 8, 512] tiles
(2 MiB per DMA), computes d = (x + eps) - y on the vector engine,
squares+row-reduces on the scalar (ACT) engine, then sqrt+reduces the
per-row norms to a [128,1] per-partition partial.  Host sums the 8x128
partials.  Memory-bound: per-core roofline ~= 32 MiB / 358 GB/s ~= 93 us.
"""

import numpy as np

import concourse.tile as tile
from concourse import bacc, bass_utils, mybir

EPS = 1e-6
N_CORES = 8
B, C, H, W = 16, 8, 512, 512
ROWS = B * C * H  # 65536 total rows of length W
ROWS_PER_CORE = ROWS // N_CORES  # 8192
P = 128  # SBUF partitions
R = 8    # rows packed per partition line (16 KiB contiguous per partition)


def _tile_schedule(rows_per_core: int, taper: bool):
    """List of (row_start, rows_per_partition) tiles covering rows_per_core.

    With taper, the final full tile is split into 4 small tiles so the
    compute tail after the last DMA lands is short.
    """
    tiles = rows_per_core // (P * R)
    assert tiles * P * R == rows_per_core
    sched = [(t * P * R, R) for t in range(tiles)]
    if taper and tiles >= 2:
        base, _ = sched.pop()
        step = R // 4
        for s in range(4):
            sched.append((base + s * P * step, step))
    return sched


def build_bass(rows_per_core: int = ROWS_PER_CORE, bufs: int = 3, loops: int = 1,
               taper: bool = True):
    """Build the per-core SPMD Bass program.

    loops > 1 repeats the streaming body (same data) for timing-by-delta;
    the output is unchanged (the repeats are idempotent).
    """
    sched = _tile_schedule(rows_per_core, taper)
    ncols = sum(r for _, r in sched)

    nc = bacc.Bacc("TRN2", target_bir_lowering=False, debug=False)
    x = nc.dram_tensor("x", [rows_per_core, W], mybir.dt.float32, kind="ExternalInput").ap()
    y = nc.dram_tensor("y", [rows_per_core, W], mybir.dt.float32, kind="ExternalInput").ap()
    out = nc.dram_tensor("out", [P, 1], mybir.dt.float32, kind="ExternalOutput").ap()

    with tile.TileContext(nc) as tc:
        with (
            tc.tile_pool(name="xp", bufs=bufs) as xp,
            tc.tile_pool(name="yp", bufs=bufs) as yp,
            tc.tile_pool(name="dp", bufs=2) as dp,
            tc.tile_pool(name="sq", bufs=2) as sqp,
            tc.tile_pool(name="st", bufs=1) as stp,
        ):
            # per-row sums of squares: one column per (tile, packed-row)
            rowsq = stp.tile([P, ncols * loops], mybir.dt.float32)
            col = 0
            for i in range(len(sched) * loops):
                start, r = sched[i % len(sched)]
                xvt = x[start:start + P * r, :].rearrange("(p r) w -> p r w", p=P, r=r)
                yvt = y[start:start + P * r, :].rearrange("(p r) w -> p r w", p=P, r=r)
                xt = xp.tile([P, R, W], mybir.dt.float32, tag="xt")
                nc.sync.dma_start(xt[:, :r, :], xvt)
                yt = yp.tile([P, R, W], mybir.dt.float32, tag="yt")
                nc.sync.dma_start(yt[:, :r, :], yvt)

                d = dp.tile([P, R, W], mybir.dt.float32, tag="d")
                # d = (x + eps) - y   (one DVE pass over the tile)
                nc.vector.scalar_tensor_tensor(
                    out=d[:, :r, :],
                    in0=xt[:, :r, :],
                    scalar=EPS,
                    in1=yt[:, :r, :],
                    op0=mybir.AluOpType.add,
                    op1=mybir.AluOpType.subtract,
                )
                # per row: sum of squares via ACT Square + free-dim accumulate
                for j in range(r):
                    sq = sqp.tile([P, W], mybir.dt.float32, tag="sq")
                    nc.scalar.activation(
                        out=sq[:],
                        in_=d[:, j, :],
                        func=mybir.ActivationFunctionType.Square,
                        accum_out=rowsq[:, col % (ncols * loops):col % (ncols * loops) + 1],
                    )
                    col += 1

            # row_norm = sqrt(rowsq); per-partition partial = sum(row_norm)
            # (columns beyond ncols stay zero when loops == 1)
            norms = stp.tile([P, ncols * loops], mybir.dt.float32)
            rowsum = stp.tile([P, 1], mybir.dt.float32)
            nc.scalar.activation(
                out=norms[:, :ncols],
                in_=rowsq[:, :ncols],
                func=mybir.ActivationFunctionType.Sqrt,
                accum_out=rowsum[:],
            )
            nc.sync.dma_start(out[:], rowsum[:])
    nc.compile()
    return nc


def build_bass_looped(loops: int):
    return build_bass(loops=loops)


_NC_CACHE = {}


def kernel(output: np.ndarray, target: np.ndarray) -> np.ndarray:
    assert output.shape == (B, C, H, W) and target.shape == (B, C, H, W)
    if "nc" not in _NC_CACHE:
        _NC_CACHE["nc"] = build_bass()
    nc = _NC_CACHE["nc"]

    X = np.ascontiguousarray(output, dtype=np.float32).reshape(N_CORES, ROWS_PER_CORE, W)
    Y = np.ascontiguousarray(target, dtype=np.float32).reshape(N_CORES, ROWS_PER_CORE, W)
    in_maps = [{"x": X[k], "y": Y[k]} for k in range(N_CORES)]
    res = bass_utils.run_bass_kernel_spmd(nc, in_maps, core_ids=list(range(N_CORES)))
    total = 0.0
    for m in res.results:
        total += float(m["out"].astype(np.float64).sum())
    return np.asarray(total, dtype=np.float32)


# revision 13
# speedup vs baseline: 54.5250x; 54.5250x over previous
"""Distance-loss kernel for Trainium2 (8 NeuronCores, SPMD data-parallel).

loss = sum_{b,c,h} || output[b,c,h,:] - target[b,c,h,:] + eps ||_2

Strategy: flatten both (16,8,512,512) f32 inputs to rows of W=512
(B*C*H = 65536 rows), shard rows contiguously across 8 cores (8192
rows/core).  Each core streams its 2 x 16 MiB in [128, 8, 512] tiles
(2 MiB per DMA), computes d = (x + eps) - y on the vector engine,
squares+row-reduces on the scalar (ACT) engine, then sqrt+reduces the
per-row norms to a [128,1] per-partition partial.  Host sums the 8x128
partials.  Memory-bound: per-core roofline ~= 32 MiB / 358 GB/s ~= 93 us.
"""

import numpy as np

import concourse.tile as tile
from concourse import bacc, bass_utils, mybir
from contextlib import ExitStack

F32 = mybir.dt.float32
NLANES = 8
NBUF = 3

EPS = 1e-6
N_CORES = 8
B, C, H, W = 16, 8, 512, 512
ROWS = B * C * H  # 65536 total rows of length W
ROWS_PER_CORE = ROWS // N_CORES  # 8192
P = 128  # SBUF partitions
R = 8    # rows packed per partition line (16 KiB contiguous per partition)


def _tile_schedule(rows_per_core: int, taper: bool):
    """List of (row_start, rows_per_partition) tiles covering rows_per_core.

    With taper, the final full tile is split into 4 small tiles so the
    compute tail after the last DMA lands is short.
    """
    tiles = rows_per_core // (P * R)
    assert tiles * P * R == rows_per_core
    sched = [(t * P * R, R) for t in range(tiles)]
    if taper and tiles >= 2:
        base, _ = sched.pop()
        step = R // 4
        for s in range(4):
            sched.append((base + s * P * step, step))
    return sched


def build_bass(rows_per_core: int = ROWS_PER_CORE, bufs: int = 3, loops: int = 1,
               taper: bool = True):
    """Build the per-core SPMD Bass program.

    loops > 1 repeats the streaming body (same data) for timing-by-delta;
    the output is unchanged (the repeats are idempotent).
    """
    sched = _tile_schedule(rows_per_core, taper)
    ncols = sum(r for _, r in sched)

    nc = bacc.Bacc("TRN2", target_bir_lowering=False, debug=False)
    x = nc.dram_tensor("x", [rows_per_core, W], mybir.dt.float32, kind="ExternalInput").ap()
    y = nc.dram_tensor("y", [rows_per_core, W], mybir.dt.float32, kind="ExternalInput").ap()
    out = nc.dram_tensor("out", [P, 1], mybir.dt.float32, kind="ExternalOutput").ap()

    with tile.TileContext(nc) as tc:
        with (
            tc.tile_pool(name="xp", bufs=bufs) as xp,
            tc.tile_pool(name="yp", bufs=bufs) as yp,
            tc.tile_pool(name="dp", bufs=2) as dp,
            tc.tile_pool(name="sq", bufs=2) as sqp,
            tc.tile_pool(name="st", bufs=1) as stp,
        ):
            # per-row sums of squares: one column per (tile, packed-row)
            rowsq = stp.tile([P, ncols * loops], mybir.dt.float32)
            col = 0
            for i in range(len(sched) * loops):
                start, r = sched[i % len(sched)]
                xvt = x[start:start + P * r, :].rearrange("(p r) w -> p r w", p=P, r=r)
                yvt = y[start:start + P * r, :].rearrange("(p r) w -> p r w", p=P, r=r)
                xt = xp.tile([P, R, W], mybir.dt.float32, tag="xt")
                nc.sync.dma_start(xt[:, :r, :], xvt)
                yt = yp.tile([P, R, W], mybir.dt.float32, tag="yt")
                nc.sync.dma_start(yt[:, :r, :], yvt)

                d = dp.tile([P, R, W], mybir.dt.float32, tag="d")
                # d = (x + eps) - y   (one DVE pass over the tile)
                nc.vector.scalar_tensor_tensor(
                    out=d[:, :r, :],
                    in0=xt[:, :r, :],
                    scalar=EPS,
                    in1=yt[:, :r, :],
                    op0=mybir.AluOpType.add,
                    op1=mybir.AluOpType.subtract,
                )
                # per row: sum of squares via ACT Square + free-dim accumulate
                for j in range(r):
                    sq = sqp.tile([P, W], mybir.dt.float32, tag="sq")
                    nc.scalar.activation(
                        out=sq[:],
                        in_=d[:, j, :],
                        func=mybir.ActivationFunctionType.Square,
                        accum_out=rowsq[:, col % (ncols * loops):col % (ncols * loops) + 1],
                    )
                    col += 1

            # row_norm = sqrt(rowsq); per-partition partial = sum(row_norm)
            # (columns beyond ncols stay zero when loops == 1)
            norms = stp.tile([P, ncols * loops], mybir.dt.float32)
            rowsum = stp.tile([P, 1], mybir.dt.float32)
            nc.scalar.activation(
                out=norms[:, :ncols],
                in_=rowsq[:, :ncols],
                func=mybir.ActivationFunctionType.Sqrt,
                accum_out=rowsum[:],
            )
            nc.sync.dma_start(out[:], rowsum[:])
    nc.compile()
    return nc


def build_bass_looped(loops: int):
    return build_bass(loops=loops)


def _raw_taper_sched(rows_per_core: int, rpp: int, taper: bool):
    tiles = rows_per_core // (P * rpp)
    assert tiles * P * rpp == rows_per_core
    sched = [(t * P * rpp, rpp) for t in range(tiles)]
    if taper and tiles >= 2 and rpp >= 4:
        base, _ = sched.pop()
        step = rpp // 4
        for s in range(4):
            sched.append((base + s * P * step, step))
    return sched


def build_raw(rows_per_core: int = ROWS_PER_CORE, rpp: int = 8, taper: bool = True,
              metaloops: int = 1):
    sched = _raw_taper_sched(rows_per_core, rpp, taper)
    ncols = sum(r for _, r in sched)
    n = len(sched)
    n_all = n * metaloops

    # --- static schedules with running counters ---------------------------
    # global tile index g = rep * n + i
    # dve tick after tile g's subtract retires: g + 1
    # act ticks: squares count 1 each, each rep's sqrt counts 1
    act_after_tile = []   # act tick after ACT finished tile g's squares
    a = 0
    for rep in range(metaloops):
        for i, (start, r) in enumerate(sched):
            a += r
            act_after_tile.append(a)
        a += 1  # the rep's sqrt
    total_act = a

    # dma lane assignment in issue order: 2 loads per tile, 1 store per rep
    lane_ticks = [0] * NLANES
    issue_lane = []  # (lane, tick-after) per dma in issue order
    k = 0
    for rep in range(metaloops):
        for i in range(n):
            for _ in range(2):
                lane = k % NLANES
                lane_ticks[lane] += 16
                issue_lane.append((lane, lane_ticks[lane]))
                k += 1
        lane = k % NLANES
        lane_ticks[lane] += 16
        issue_lane.append((lane, lane_ticks[lane]))
        k += 1

    nc = bacc.Bacc("TRN2", target_bir_lowering=False, debug=False)
    x = nc.dram_tensor("x", [rows_per_core, W], F32, kind="ExternalInput").ap()
    y = nc.dram_tensor("y", [rows_per_core, W], F32, kind="ExternalInput").ap()
    out = nc.dram_tensor("out", [P, 1], F32, kind="ExternalOutput").ap()

    def view(ap, start, r):
        return ap[start:start + P * r, :].rearrange("(p r) w -> p (r w)", p=P, r=r)

    with ExitStack() as ctx:
        sb = lambda name, shape: ctx.enter_context(nc.sbuf_tensor(name, shape, F32))
        sem = lambda name: ctx.enter_context(nc.semaphore(name))
        xs = [sb(f"xb{j}", [P, rpp * W]) for j in range(NBUF)]
        ys = [sb(f"yb{j}", [P, rpp * W]) for j in range(NBUF)]
        ds = [sb(f"db{j}", [P, rpp * W]) for j in range(NBUF)]
        sqbs = [sb("sqb0", [P, W]), sb("sqb1", [P, W])]
        rowsq = sb("rowsq", [P, ncols])
        norms = sb("norms", [P, ncols])
        rowsum = sb("rowsum", [P, 1])
        lanes = [sem(f"dma{j}") for j in range(NLANES)]
        dve_sem = sem("dve_sem")
        act_sem = sem("act_sem")
        block = ctx.enter_context(nc.Block())

        @block.sync
        def _(sync):
            dma_idx = 0
            for rep in range(metaloops):
                for i, (start, r) in enumerate(sched):
                    g = rep * n + i
                    if g >= NBUF:
                        # x/y slot g%NBUF frees when subtract g-NBUF retires
                        sync.wait_ge(dve_sem, g - NBUF + 1)
                    lx, _ = issue_lane[dma_idx]
                    ly, _ = issue_lane[dma_idx + 1]
                    sync.dma_start(xs[g % NBUF][:, :r * W], view(x, start, r)).then_inc(lanes[lx], 16)
                    sync.dma_start(ys[g % NBUF][:, :r * W], view(y, start, r)).then_inc(lanes[ly], 16)
                    dma_idx += 2
                # this rep's sqrt done -> rowsum valid
                sync.wait_ge(act_sem, act_after_tile[rep * n + n - 1] + 1)
                lo, to = issue_lane[dma_idx]
                sync.dma_start(out, rowsum[:]).then_inc(lanes[lo], 16)
                sync.wait_ge(lanes[lo], to)
                dma_idx += 1
            # program end: observe every sem's final value so nothing is in
            # flight when the engines run off the end of their queues
            for lane_idx, s in enumerate(lanes):
                if lane_ticks[lane_idx]:
                    sync.wait_ge(s, lane_ticks[lane_idx])
            sync.wait_ge(dve_sem, n_all)

        @block.vector
        def _(vector):
            dma_idx = 0
            for rep in range(metaloops):
                for i, (start, r) in enumerate(sched):
                    g = rep * n + i
                    lx, tx = issue_lane[dma_idx]
                    ly, ty = issue_lane[dma_idx + 1]
                    dma_idx += 2
                    vector.wait_ge(lanes[lx], tx)
                    vector.wait_ge(lanes[ly], ty)
                    if g >= NBUF:
                        # d slot g%NBUF frees when ACT finished tile g-NBUF
                        vector.wait_ge(act_sem, act_after_tile[g - NBUF])
                    nc.vector.scalar_tensor_tensor(
                        out=ds[g % NBUF][:, :r * W],
                        in0=xs[g % NBUF][:, :r * W],
                        scalar=EPS,
                        in1=ys[g % NBUF][:, :r * W],
                        op0=mybir.AluOpType.add,
                        op1=mybir.AluOpType.subtract,
                    ).then_inc(dve_sem, 1)
                dma_idx += 1  # skip the rep's out store

        @block.scalar
        def _(scalar):
            a = 0  # running act tick
            for rep in range(metaloops):
                col = 0
                for i, (start, r) in enumerate(sched):
                    g = rep * n + i
                    scalar.wait_ge(dve_sem, g + 1)
                    for j in range(r):
                        if a >= 2:
                            # scratch slot written two ACT ops ago must have
                            # retired; act_sem >= a-1 is already true at issue
                            scalar.wait_ge(act_sem, a - 1)
                        nc.scalar.activation(
                            out=sqbs[a % 2][:],
                            in_=ds[g % NBUF][:, j * W:(j + 1) * W],
                            func=mybir.ActivationFunctionType.Square,
                            accum_out=rowsq[:, col:col + 1],
                        ).then_inc(act_sem, 1)
                        a += 1
                        col += 1
                # all this rep's squares retired before the sqrt reads rowsq
                scalar.wait_ge(act_sem, a)
                nc.scalar.activation(
                    out=norms[:],
                    in_=rowsq[:],
                    func=mybir.ActivationFunctionType.Sqrt,
                    accum_out=rowsum[:],
                ).then_inc(act_sem, 1)
                a += 1

    nc.compile()
    return nc


_NC_CACHE = {}


def kernel(output: np.ndarray, target: np.ndarray) -> np.ndarray:
    assert output.shape == (B, C, H, W) and target.shape == (B, C, H, W)
    if "nc" not in _NC_CACHE:
        _NC_CACHE["nc"] = build_raw()
    nc = _NC_CACHE["nc"]

    X = np.ascontiguousarray(output, dtype=np.float32).reshape(N_CORES, ROWS_PER_CORE, W)
    Y = np.ascontiguousarray(target, dtype=np.float32).reshape(N_CORES, ROWS_PER_CORE, W)
    in_maps = [{"x": X[k], "y": Y[k]} for k in range(N_CORES)]
    res = bass_utils.run_bass_kernel_spmd(nc, in_maps, core_ids=list(range(N_CORES)))
    total = 0.0
    for m in res.results:
        total += float(m["out"].astype(np.float64).sum())
    return np.asarray(total, dtype=np.float32)


# revision 20
# speedup vs baseline: 60.8673x; 1.1163x over previous
"""Distance-loss kernel for Trainium2 (8 NeuronCores, SPMD data-parallel).

loss = sum_{b,c,h} || output[b,c,h,:] - target[b,c,h,:] + eps ||_2

Strategy: flatten both (16,8,512,512) f32 inputs to rows of W=512
(B*C*H = 65536 rows), shard rows contiguously across 8 cores (8192
rows/core).  Each core streams its 2 x 16 MiB in [128, 8, 512] tiles
(2 MiB per DMA), computes d = (x + eps) - y on the vector engine and
per-row sums of squares on the scalar (ACT) engine, writing rowsq
[128, 64] per core.  The host finishes in float64: sqrt per row, sum.
Memory-bound: per-core roofline ~= 32 MiB / ~390 GB/s ~= 85-95 us.
"""

import numpy as np

import concourse.tile as tile
from concourse import bacc, bass_utils, mybir
from contextlib import ExitStack

F32 = mybir.dt.float32
NLANES = 8
NBUF = 3

EPS = 1e-6
N_CORES = 8
B, C, H, W = 16, 8, 512, 512
ROWS = B * C * H  # 65536 total rows of length W
ROWS_PER_CORE = ROWS // N_CORES  # 8192
P = 128  # SBUF partitions
R = 8    # rows packed per partition line (16 KiB contiguous per partition)


def _tile_schedule(rows_per_core: int, taper: bool):
    """List of (row_start, rows_per_partition) tiles covering rows_per_core.

    With taper, the final full tile is split into 4 small tiles so the
    compute tail after the last DMA lands is short.
    """
    tiles = rows_per_core // (P * R)
    assert tiles * P * R == rows_per_core
    sched = [(t * P * R, R) for t in range(tiles)]
    if taper and tiles >= 2:
        base, _ = sched.pop()
        step = R // 4
        for s in range(4):
            sched.append((base + s * P * step, step))
    return sched


def build_bass(rows_per_core: int = ROWS_PER_CORE, bufs: int = 3, loops: int = 1,
               taper: bool = True):
    """Build the per-core SPMD Bass program.

    loops > 1 repeats the streaming body (same data) for timing-by-delta;
    the output is unchanged (the repeats are idempotent).
    """
    sched = _tile_schedule(rows_per_core, taper)
    ncols = sum(r for _, r in sched)

    nc = bacc.Bacc("TRN2", target_bir_lowering=False, debug=False)
    x = nc.dram_tensor("x", [rows_per_core, W], mybir.dt.float32, kind="ExternalInput").ap()
    y = nc.dram_tensor("y", [rows_per_core, W], mybir.dt.float32, kind="ExternalInput").ap()
    out = nc.dram_tensor("out", [P, 1], mybir.dt.float32, kind="ExternalOutput").ap()

    with tile.TileContext(nc) as tc:
        with (
            tc.tile_pool(name="xp", bufs=bufs) as xp,
            tc.tile_pool(name="yp", bufs=bufs) as yp,
            tc.tile_pool(name="dp", bufs=2) as dp,
            tc.tile_pool(name="sq", bufs=2) as sqp,
            tc.tile_pool(name="st", bufs=1) as stp,
        ):
            # per-row sums of squares: one column per (tile, packed-row)
            rowsq = stp.tile([P, ncols * loops], mybir.dt.float32)
            col = 0
            for i in range(len(sched) * loops):
                start, r = sched[i % len(sched)]
                xvt = x[start:start + P * r, :].rearrange("(p r) w -> p r w", p=P, r=r)
                yvt = y[start:start + P * r, :].rearrange("(p r) w -> p r w", p=P, r=r)
                xt = xp.tile([P, R, W], mybir.dt.float32, tag="xt")
                nc.sync.dma_start(xt[:, :r, :], xvt)
                yt = yp.tile([P, R, W], mybir.dt.float32, tag="yt")
                nc.sync.dma_start(yt[:, :r, :], yvt)

                d = dp.tile([P, R, W], mybir.dt.float32, tag="d")
                # d = (x + eps) - y   (one DVE pass over the tile)
                nc.vector.scalar_tensor_tensor(
                    out=d[:, :r, :],
                    in0=xt[:, :r, :],
                    scalar=EPS,
                    in1=yt[:, :r, :],
                    op0=mybir.AluOpType.add,
                    op1=mybir.AluOpType.subtract,
                )
                # per row: sum of squares via ACT Square + free-dim accumulate
                for j in range(r):
                    sq = sqp.tile([P, W], mybir.dt.float32, tag="sq")
                    nc.scalar.activation(
                        out=sq[:],
                        in_=d[:, j, :],
                        func=mybir.ActivationFunctionType.Square,
                        accum_out=rowsq[:, col % (ncols * loops):col % (ncols * loops) + 1],
                    )
                    col += 1

            # row_norm = sqrt(rowsq); per-partition partial = sum(row_norm)
            # (columns beyond ncols stay zero when loops == 1)
            norms = stp.tile([P, ncols * loops], mybir.dt.float32)
            rowsum = stp.tile([P, 1], mybir.dt.float32)
            nc.scalar.activation(
                out=norms[:, :ncols],
                in_=rowsq[:, :ncols],
                func=mybir.ActivationFunctionType.Sqrt,
                accum_out=rowsum[:],
            )
            nc.sync.dma_start(out[:], rowsum[:])
    nc.compile()
    return nc


def build_bass_looped(loops: int):
    return build_bass(loops=loops)


def _raw_taper_sched(rows_per_core: int, rpp: int, taper: bool):
    tiles = rows_per_core // (P * rpp)
    assert tiles * P * rpp == rows_per_core
    sched = [(t * P * rpp, rpp) for t in range(tiles)]
    if taper and tiles >= 2 and rpp >= 4:
        base, _ = sched.pop()
        step = rpp // 4
        for s in range(4):
            sched.append((base + s * P * step, step))
    return sched


def build_raw(rows_per_core: int = ROWS_PER_CORE, rpp: int = 8, taper: bool = True,
              metaloops: int = 1, halves: int = 1):
    """Device program: per row of W=512, the sum of squares of
    d = (x + eps) - y, accumulated in f32 on the ACT engine.  Output is
    rowsq [P, halves*ncols] (with halves > 1, each row is accumulated in
    `halves` independent chunks for shorter f32 accumulation chains).
    sqrt + final sum happen on the host in float64 -- removing the device
    sqrt and its ACT-table switch from the critical path.
    """
    sched = _raw_taper_sched(rows_per_core, rpp, taper)
    ncols = sum(r for _, r in sched)
    n = len(sched)
    n_all = n * metaloops
    HALF = W // halves

    # --- static schedules with running counters ---------------------------
    # global tile index g = rep * n + i
    # dve tick after tile g's subtract retires: g + 1
    # act ticks: two half-squares per row
    act_after_tile = []   # act tick after ACT finished tile g's squares
    a = 0
    for rep in range(metaloops):
        for i, (start, r) in enumerate(sched):
            a += halves * r
            act_after_tile.append(a)
    total_act = a

    # dma lane assignment in issue order: 2 loads per tile, 1 store per rep
    lane_ticks = [0] * NLANES
    issue_lane = []  # (lane, tick-after) per dma in issue order
    k = 0
    for rep in range(metaloops):
        for i in range(n):
            for _ in range(2):
                lane = k % NLANES
                lane_ticks[lane] += 16
                issue_lane.append((lane, lane_ticks[lane]))
                k += 1
        lane = k % NLANES
        lane_ticks[lane] += 16
        issue_lane.append((lane, lane_ticks[lane]))
        k += 1

    nc = bacc.Bacc("TRN2", target_bir_lowering=False, debug=False)
    x = nc.dram_tensor("x", [rows_per_core, W], F32, kind="ExternalInput").ap()
    y = nc.dram_tensor("y", [rows_per_core, W], F32, kind="ExternalInput").ap()
    out = nc.dram_tensor("out", [P, halves * ncols], F32, kind="ExternalOutput").ap()

    def view(ap, start, r):
        return ap[start:start + P * r, :].rearrange("(p r) w -> p (r w)", p=P, r=r)

    with ExitStack() as ctx:
        sb = lambda name, shape: ctx.enter_context(nc.sbuf_tensor(name, shape, F32))
        sem = lambda name: ctx.enter_context(nc.semaphore(name))
        xs = [sb(f"xb{j}", [P, rpp * W]) for j in range(NBUF)]
        ys = [sb(f"yb{j}", [P, rpp * W]) for j in range(NBUF)]
        ds = [sb(f"db{j}", [P, rpp * W]) for j in range(NBUF)]
        sqbs = [sb(f"sqb{j}", [P, HALF]) for j in range(3)]
        rowsq = sb("rowsq", [P, halves * ncols])
        lanes = [sem(f"dma{j}") for j in range(NLANES)]
        dve_sem = sem("dve_sem")
        act_sem = sem("act_sem")
        block = ctx.enter_context(nc.Block())

        @block.sync
        def _(sync):
            dma_idx = 0
            for rep in range(metaloops):
                for i, (start, r) in enumerate(sched):
                    g = rep * n + i
                    if g >= NBUF:
                        # x/y slot g%NBUF frees when subtract g-NBUF retires
                        sync.wait_ge(dve_sem, g - NBUF + 1)
                    lx, _ = issue_lane[dma_idx]
                    ly, _ = issue_lane[dma_idx + 1]
                    sync.dma_start(xs[g % NBUF][:, :r * W], view(x, start, r)).then_inc(lanes[lx], 16)
                    sync.dma_start(ys[g % NBUF][:, :r * W], view(y, start, r)).then_inc(lanes[ly], 16)
                    dma_idx += 2
                # this rep's squares all retired -> rowsq valid
                sync.wait_ge(act_sem, act_after_tile[rep * n + n - 1])
                lo, to = issue_lane[dma_idx]
                sync.dma_start(out, rowsq[:]).then_inc(lanes[lo], 16)
                sync.wait_ge(lanes[lo], to)
                dma_idx += 1
            # program end: observe every sem's final value so nothing is in
            # flight when the engines run off the end of their queues
            for lane_idx, s in enumerate(lanes):
                if lane_ticks[lane_idx]:
                    sync.wait_ge(s, lane_ticks[lane_idx])
            sync.wait_ge(dve_sem, n_all)

        @block.vector
        def _(vector):
            dma_idx = 0
            for rep in range(metaloops):
                for i, (start, r) in enumerate(sched):
                    g = rep * n + i
                    lx, tx = issue_lane[dma_idx]
                    ly, ty = issue_lane[dma_idx + 1]
                    dma_idx += 2
                    vector.wait_ge(lanes[lx], tx)
                    vector.wait_ge(lanes[ly], ty)
                    if g >= NBUF:
                        # d slot g%NBUF frees when ACT finished tile g-NBUF
                        vector.wait_ge(act_sem, act_after_tile[g - NBUF])
                    nc.vector.scalar_tensor_tensor(
                        out=ds[g % NBUF][:, :r * W],
                        in0=xs[g % NBUF][:, :r * W],
                        scalar=EPS,
                        in1=ys[g % NBUF][:, :r * W],
                        op0=mybir.AluOpType.add,
                        op1=mybir.AluOpType.subtract,
                    ).then_inc(dve_sem, 1)
                dma_idx += 1  # skip the rep's out store

        @block.scalar
        def _(scalar):
            a = 0  # running act tick
            for rep in range(metaloops):
                col = 0
                for i, (start, r) in enumerate(sched):
                    g = rep * n + i
                    scalar.wait_ge(dve_sem, g + 1)
                    for j in range(r):
                        for h in range(halves):
                            if a >= 3:
                                # scratch slot written three ACT ops ago must
                                # have retired; act_sem >= a-2 is already
                                # true at issue, so no stall in steady state
                                scalar.wait_ge(act_sem, a - 2)
                            nc.scalar.activation(
                                out=sqbs[a % 3][:],
                                in_=ds[g % NBUF][:, j * W + h * HALF:j * W + (h + 1) * HALF],
                                func=mybir.ActivationFunctionType.Square,
                                accum_out=rowsq[:, h * ncols + col:h * ncols + col + 1],
                            ).then_inc(act_sem, 1)
                            a += 1
                        col += 1

    nc.compile()
    return nc


_NC_CACHE = {}


def kernel(output: np.ndarray, target: np.ndarray) -> np.ndarray:
    assert output.shape == (B, C, H, W) and target.shape == (B, C, H, W)
    if "nc" not in _NC_CACHE:
        _NC_CACHE["nc"] = build_raw()
    nc = _NC_CACHE["nc"]

    X = np.ascontiguousarray(output, dtype=np.float32).reshape(N_CORES, ROWS_PER_CORE, W)
    Y = np.ascontiguousarray(target, dtype=np.float32).reshape(N_CORES, ROWS_PER_CORE, W)
    in_maps = [{"x": X[k], "y": Y[k]} for k in range(N_CORES)]

    # The tunneled device occasionally comes up wedged from a previous
    # process (NRT_EXEC_UNIT_UNRECOVERABLE on the first execution).  A
    # backend reset + retry recovers it.
    last_err = None
    for attempt in range(3):
        try:
            if attempt > 0:
                import time
                time.sleep(5 * attempt)
                try:
                    import jax
                    jax.clear_caches()
                    jax.extend.backend.clear_backends()
                except Exception:
                    pass
            res = bass_utils.run_bass_kernel_spmd(nc, in_maps, core_ids=list(range(N_CORES)))
            break
        except Exception as e:  # noqa: BLE001
            last_err = e
    else:
        raise last_err

    # host finish in float64: per-row sum of squares -> norm -> scalar sum
    # (device emits rowsq [P, ncols], one column per row; halves == 1)
    total = 0.0
    for m in res.results:
        rowsq = m["out"].astype(np.float64)
        total += float(np.sqrt(rowsq).sum())
    return np.asarray(total, dtype=np.float32)
